# revision 12
# baseline (speedup 1.0000x reference)
"""Energy Transformer (ET) Trainium2 kernel.

Data-parallel over batch: 32 samples -> 8 cores x 4 samples. Parameters
replicated; no collectives (inference only).

Math (verified against jax.grad to 6e-7):
  out += sum_h [ (P K) Wq_h^T + (P^T Q) Wk_h^T ] + relu(g xi) xi^T
  where g = energy-LN(out), P = softmax_k(beta Q K^T) per (sample, head).

Token order on device: [patch 0..195, cls] per sample (attention/LN are
permutation-invariant; pos-emb rows are pre-permuted on host). This keeps
the patch-embedding matmul partition-aligned with the residual tiles.

On-chip layouts per core (4 samples, 197 tokens each, TOK=788):
  out  : token-major fp32 residual, per-sample tiles (128|69, 768)
  gT   : feature-major bf16 (6 x (128, 788)) - matmul operand
  QT/KT: e-major bf16 (6 head-pair tiles x (128, 788))
  P^T is produced directly as exp(beta*S^T - lnZ) with the per-column lnZ
  injected via a rank-1 matmul accumulation (ones^T @ zrow).
"""

import sys
for _p in ("/opt/trn_rl_repo",):
    if _p not in sys.path:
        sys.path.insert(0, _p)

import numpy as np
import ml_dtypes
from contextlib import ExitStack

import concourse.bass as bass
import concourse.bacc as bacc
import concourse.mybir as mybir
import concourse.tile as tile
from concourse.bass_utils import run_bass_kernel_spmd

BF = mybir.dt.bfloat16
F32 = mybir.dt.float32
AF = mybir.ActivationFunctionType

# ---- problem dims ----
N_CORES = 8
B = 32
SPC = B // N_CORES      # samples per core = 4
IMG = 224
PATCH = 14
PS = IMG // PATCH       # 16
F = PS * PS * 3         # 768
D = 768
DC = D // 128           # 6
H = 12
HH = H // 2             # 6 head-pairs
DK = 64
HN = 4 * D              # 3072
MC = HN // 128          # 24
NT = PATCH * PATCH + 1  # 197
NPATCH = NT - 1         # 196
TOK = SPC * NT          # 788
HALF = TOK // 2         # 394
NCLS = 1000
N_RECUR = 7
BETA = 1.0 / float(np.sqrt(DK))
EPS = 1e-5

QSZ = (128, NT - 128)      # per-sample token chunks: 128, 69
CLS_ROW = NPATCH - 128     # row 68 of chunk 1 holds the cls token

USE_LOOP = True            # tc.For_i over recurrence steps vs full unroll
N_ITER = N_RECUR


def build_nc(n_iter=N_ITER, use_loop=USE_LOOP):
    nc = bacc.Bacc(None, target_bir_lowering=False)

    # ---- DRAM I/O ----
    d_pT = nc.dram_tensor("pT", [F, SPC * NPATCH], BF, kind="ExternalInput")
    d_embwT = nc.dram_tensor("embwT", [F, D], BF, kind="ExternalInput")
    d_posc = nc.dram_tensor("posc", [NT, D], F32, kind="ExternalInput")
    d_wq_De = nc.dram_tensor("wq_De", [D, D], BF, kind="ExternalInput")
    d_wk_De = nc.dram_tensor("wk_De", [D, D], BF, kind="ExternalInput")
    d_wq_eD = nc.dram_tensor("wq_eD", [D, D], BF, kind="ExternalInput")
    d_wk_eD = nc.dram_tensor("wk_eD", [D, D], BF, kind="ExternalInput")
    d_xiblk = nc.dram_tensor("xiblk", [DC, MC, 128, 128], BF, kind="ExternalInput")
    d_ximD = nc.dram_tensor("ximD", [HN, D], BF, kind="ExternalInput")
    d_delta = nc.dram_tensor("delta_bc", [128, D], BF, kind="ExternalInput")
    d_gamma = nc.dram_tensor("gamma_bc", [128, 1], F32, kind="ExternalInput")
    d_ident = nc.dram_tensor("ident_bf", [128, 128], BF, kind="ExternalInput")
    d_ones = nc.dram_tensor("ones_bf", [1, 128], BF, kind="ExternalInput")
    d_lnw = nc.dram_tensor("lnw_bc", [SPC, D], F32, kind="ExternalInput")
    d_lnb = nc.dram_tensor("lnb_bc", [SPC, D], F32, kind="ExternalInput")
    d_fcwT = nc.dram_tensor("fcwT", [D, NCLS], BF, kind="ExternalInput")
    d_fcb = nc.dram_tensor("fcb_bc", [SPC, NCLS], F32, kind="ExternalInput")
    d_y = nc.dram_tensor("y", [SPC, NCLS], F32, kind="ExternalOutput")

    with ExitStack() as ctx:
        tc = ctx.enter_context(tile.TileContext(nc))
        consts = ctx.enter_context(tc.tile_pool(name="consts", bufs=1))
        state = ctx.enter_context(tc.tile_pool(name="state", bufs=1))

        def ptiles(pool, n, shape, dtype, tag):
            return [pool.tile(shape, dtype, tag=f"{tag}{i}", name=f"{tag}{i}") for i in range(n)]

        # ---- resident constants ----
        wq_De = ptiles(consts, DC, [128, D], BF, "wq_De")
        wk_De = ptiles(consts, DC, [128, D], BF, "wk_De")
        wq_eD = ptiles(consts, HH, [128, D], BF, "wq_eD")
        wk_eD = ptiles(consts, HH, [128, D], BF, "wk_eD")
        xi_mD = ptiles(consts, MC, [128, D], BF, "xi_mD")
        delta_bc = consts.tile([128, D], BF, tag="delta_bc")
        gamma_bc = consts.tile([128, 1], F32, tag="gamma_bc")
        eps_t = consts.tile([128, 1], F32, tag="eps_t")
        ident = consts.tile([128, 128], BF, tag="ident")
        ones_r = consts.tile([1, 128], BF, tag="ones_r")

        for i in range(DC):
            nc.sync.dma_start(out=wq_De[i], in_=d_wq_De[i * 128:(i + 1) * 128, :])
            nc.sync.dma_start(out=wk_De[i], in_=d_wk_De[i * 128:(i + 1) * 128, :])
        for i in range(HH):
            nc.sync.dma_start(out=wq_eD[i], in_=d_wq_eD[i * 128:(i + 1) * 128, :])
            nc.sync.dma_start(out=wk_eD[i], in_=d_wk_eD[i * 128:(i + 1) * 128, :])
        for i in range(MC):
            nc.sync.dma_start(out=xi_mD[i], in_=d_ximD[i * 128:(i + 1) * 128, :])
        nc.sync.dma_start(out=delta_bc, in_=d_delta[:, :])
        nc.sync.dma_start(out=gamma_bc, in_=d_gamma[:, :])
        nc.sync.dma_start(out=ident, in_=d_ident[:, :])
        nc.sync.dma_start(out=ones_r, in_=d_ones[:, :])
        nc.vector.memset(eps_t, EPS)

        # ---- persistent state ----
        out_t = []  # [2*s + c]
        for s in range(SPC):
            for c in range(2):
                out_t.append(state.tile([QSZ[c], D], F32, tag=f"out_{s}_{c}", name=f"out_{s}_{c}"))
        gT = ptiles(state, DC, [128, TOK], BF, "gT")
        QT = ptiles(state, HH, [128, TOK], BF, "QT")
        KT = ptiles(state, HH, [128, TOK], BF, "KT")
        U1T = ptiles(state, HH, [128, TOK], BF, "U1T")
        T2T = ptiles(state, HH, [128, TOK], BF, "T2T")
        UTc = ptiles(state, DC, [128, TOK], BF, "UTc")

        # ================= INIT: patch embedding =================
        # token j<196 is patch j; token 196 is cls (posc pre-permuted on host)
        with tc.tile_pool(name="init", bufs=1) as initp, \
             tc.tile_pool(name="init_ps", bufs=4, space="PSUM") as ipsum:
            pT_sb = ptiles(initp, DC, [128, SPC * NPATCH], BF, "pT")
            embT_sb = ptiles(initp, DC, [128, D], BF, "embT")
            posc_sb = [initp.tile([QSZ[c], D], F32, tag=f"posc{c}", name=f"posc{c}") for c in range(2)]
            for i in range(DC):
                nc.sync.dma_start(out=pT_sb[i], in_=d_pT[i * 128:(i + 1) * 128, :])
                nc.sync.dma_start(out=embT_sb[i], in_=d_embwT[i * 128:(i + 1) * 128, :])
            nc.sync.dma_start(out=posc_sb[0], in_=d_posc[0:128, :])
            nc.sync.dma_start(out=posc_sb[1], in_=d_posc[128:NT, :])

            for s in range(SPC):
                for pc in range(2):
                    psz = (128, CLS_ROW)[pc]   # 128 / 68 patches
                    for nh in range(2):
                        nsl = slice(nh * 384, (nh + 1) * 384)
                        ps = ipsum.tile([128, 384], F32, tag="emb_ps")
                        for fi in range(DC):
                            nc.tensor.matmul(
                                ps[:psz],
                                pT_sb[fi][:, s * NPATCH + pc * 128:
                                          s * NPATCH + pc * 128 + psz],
                                embT_sb[fi][:, nsl],
                                start=(fi == 0), stop=(fi == DC - 1))
                        nc.vector.tensor_add(
                            out_t[2 * s + pc][0:psz, nsl], ps[0:psz, :],
                            posc_sb[pc][0:psz, nsl])
                # cls token row: chunk1 row CLS_ROW (no patch contribution)
                nc.sync.dma_start(out=out_t[2 * s + 1][CLS_ROW:CLS_ROW + 1, :],
                                  in_=posc_sb[1][CLS_ROW:CLS_ROW + 1, :])

        # ================= RECURRENCE =================
        with tc.tile_pool(name="lnp", bufs=2) as lnp, \
             tc.tile_pool(name="gtk", bufs=2) as gtk, \
             tc.tile_pool(name="atk", bufs=1) as atk, \
             tc.tile_pool(name="atn", bufs=4) as atn, \
             tc.tile_pool(name="hidp", bufs=1) as hidp, \
             tc.tile_pool(name="xip", bufs=8) as xip, \
             tc.tile_pool(name="ps_t", bufs=2, space="PSUM") as ps_t, \
             tc.tile_pool(name="ps_mm", bufs=2, space="PSUM") as ps_mm, \
             tc.tile_pool(name="ps_at", bufs=2, space="PSUM") as ps_at, \
             tc.tile_pool(name="ps_sm", bufs=2, space="PSUM") as ps_sm:

            def iteration_body():
                # ---- LN + transpose g ----
                for s in range(SPC):
                    for c in range(2):
                        qsz = QSZ[c]
                        ot = out_t[2 * s + c]
                        stats = lnp.tile([128, 3, 6], F32, tag="bnstats")
                        mv = lnp.tile([128, 2], F32, tag="mv")
                        for sg in range(3):
                            nc.vector.bn_stats(out=stats[:qsz, sg, :],
                                               in_=ot[:qsz, sg * 256:(sg + 1) * 256])
                        nc.vector.bn_aggr(out=mv[:qsz], in_=stats[:qsz])
                        rstd = lnp.tile([128, 1], F32, tag="rstd")
                        nc.scalar.activation(out=rstd[:qsz], in_=mv[:qsz, 1:2],
                                             func=AF.Sqrt, bias=eps_t[:qsz], scale=1.0)
                        nc.vector.reciprocal(rstd[:qsz], rstd[:qsz])
                        nc.vector.tensor_mul(rstd[:qsz], rstd[:qsz], gamma_bc[:qsz])
                        negmu = lnp.tile([128, 1], F32, tag="negmu")
                        nc.vector.tensor_mul(negmu[:qsz], mv[:qsz, 0:1], rstd[:qsz])
                        nc.vector.tensor_scalar_mul(negmu[:qsz], negmu[:qsz], -1.0)
                        g1 = lnp.tile([128, D], F32, tag="g1")
                        nc.scalar.activation(out=g1[:qsz], in_=ot[:qsz],
                                             func=AF.Identity,
                                             bias=negmu[:qsz], scale=rstd[:qsz])
                        gtok = gtk.tile([128, D], BF, tag="gtok")
                        nc.vector.tensor_add(gtok[:qsz], g1[:qsz], delta_bc[:qsz])
                        col = s * NT + c * 128
                        for d in range(DC):
                            pst = ps_t.tile([128, 128], BF, tag="pst")
                            nc.tensor.transpose(pst[:, :qsz],
                                                gtok[:qsz, d * 128:(d + 1) * 128],
                                                ident[:qsz, :qsz])
                            nc.scalar.copy(out=gT[d][:, col:col + qsz],
                                           in_=pst[:, :qsz])

                # ---- Q/K projections (e-major) ----
                for (W, OUT) in ((wq_De, QT), (wk_De, KT)):
                    for hh in range(HH):
                        for hf in range(2):
                            fsl = slice(hf * HALF, (hf + 1) * HALF)
                            ps = ps_mm.tile([128, HALF], F32, tag="proj_ps")
                            for d in range(DC):
                                nc.tensor.matmul(
                                    ps, W[d][:, hh * 128:(hh + 1) * 128],
                                    gT[d][:, fsl],
                                    start=(d == 0), stop=(d == DC - 1))
                            nc.scalar.copy(out=OUT[hh][:, fsl], in_=ps)

                # ---- attention per sample ----
                for s in range(SPC):
                    scol = slice(s * NT, (s + 1) * NT)
                    # token-major Q/K for this sample
                    Qs = [atk.tile([128, D], BF, tag=f"Qs{c}", name=f"Qs{c}") for c in range(2)]
                    Ks = [atk.tile([128, D], BF, tag=f"Ks{c}", name=f"Ks{c}") for c in range(2)]
                    for (SRC, DST) in ((QT, Qs), (KT, Ks)):
                        for c in range(2):
                            qsz = QSZ[c]
                            col = s * NT + c * 128
                            for hh in range(HH):
                                pst = ps_t.tile([128, 128], BF, tag="pst")
                                nc.tensor.transpose(pst[:qsz, :],
                                                    SRC[hh][:, col:col + qsz], ident)
                                nc.scalar.copy(
                                    out=DST[c][:qsz, hh * 128:(hh + 1) * 128],
                                    in_=pst[:qsz, :])
                    for h in range(H):
                        hh, ho = h // 2, (h % 2) * 64
                        esl = slice(ho, ho + 64)
                        hsl = slice(h * 64, (h + 1) * 64)
                        # E-side: S, exp, Z, P; also -lnZ/beta
                        Es, nlz = [], []
                        for c in range(2):
                            qsz = QSZ[c]
                            col = s * NT + c * 128
                            ps = ps_at.tile([128, NT], F32, tag="psS")
                            nc.tensor.matmul(ps[:qsz], QT[hh][esl, col:col + qsz],
                                             KT[hh][esl, scol], start=True, stop=True)
                            E = atn.tile([128, NT], BF, tag="E")
                            Z = atn.tile([128, 1], F32, tag="Z")
                            nc.scalar.activation(out=E[:qsz], in_=ps[:qsz],
                                                 func=AF.Exp, scale=BETA,
                                                 accum_out=Z[:qsz])
                            Zi = atn.tile([128, 1], F32, tag="Zi")
                            nc.vector.reciprocal(Zi[:qsz], Z[:qsz])
                            nl = atn.tile([128, 1], BF, tag="nl")
                            nc.scalar.activation(out=nl[:qsz], in_=Z[:qsz],
                                                 func=AF.Ln, scale=1.0)
                            nc.vector.tensor_scalar_mul(nl[:qsz], nl[:qsz],
                                                        -1.0 / BETA)
                            # P = E * (1/Z) in place
                            nc.vector.tensor_scalar_mul(E[:qsz], E[:qsz], Zi[:qsz])
                            Es.append(E)
                            nlz.append(nl)
                        # T2^T = Q^T P  (e-major out)
                        psT2 = ps_sm.tile([64, NT], F32, tag="ps_sm")
                        for c in range(2):
                            qsz = QSZ[c]
                            nc.tensor.matmul(psT2, Qs[c][:qsz, hsl], Es[c][:qsz],
                                             start=(c == 0), stop=(c == 1))
                        nc.vector.tensor_copy(out=T2T[hh][esl, scol], in_=psT2)
                        # zrow = transpose(-lnZ/beta)
                        zr = atn.tile([1, NT], BF, tag="zr")
                        for c in range(2):
                            qsz = QSZ[c]
                            psz_t = ps_t.tile([128, 128], BF, tag="pst")
                            nc.tensor.transpose(psz_t[:1, :qsz], nlz[c][:qsz, :],
                                                ident[:qsz, :qsz])
                            nc.scalar.copy(out=zr[:, c * 128:c * 128 + qsz],
                                           in_=psz_t[:1, :qsz])
                        # ST-side: S^T with -lnZ bias folded in; exp -> P^T
                        PTs = []
                        for c in range(2):
                            ksz = QSZ[c]
                            col = s * NT + c * 128
                            ps = ps_at.tile([128, NT], F32, tag="psS")
                            nc.tensor.matmul(ps[:ksz], KT[hh][esl, col:col + ksz],
                                             QT[hh][esl, scol], start=True, stop=False)
                            nc.tensor.matmul(ps[:ksz], ones_r[:1, :ksz], zr,
                                             start=False, stop=True)
                            PT = atn.tile([128, NT], BF, tag="PT")
                            nc.scalar.activation(out=PT[:ksz], in_=ps[:ksz],
                                                 func=AF.Exp, scale=BETA)
                            PTs.append(PT)
                        # U1 = K^T P^T (normalized T1^T, e-major)
                        psU = ps_sm.tile([64, NT], F32, tag="ps_sm")
                        for c in range(2):
                            ksz = QSZ[c]
                            nc.tensor.matmul(psU, Ks[c][:ksz, hsl], PTs[c][:ksz],
                                             start=(c == 0), stop=(c == 1))
                        nc.vector.tensor_copy(out=U1T[hh][esl, scol], in_=psU)

                # ---- attention output projections ----
                for d in range(DC):
                    dsl = slice(d * 128, (d + 1) * 128)
                    for hf in range(2):
                        fsl = slice(hf * HALF, (hf + 1) * HALF)
                        ps1 = ps_mm.tile([128, HALF], F32, tag="proj_ps")
                        for hh in range(HH):
                            nc.tensor.matmul(ps1, wq_eD[hh][:, dsl], U1T[hh][:, fsl],
                                             start=(hh == 0), stop=(hh == HH - 1))
                        nc.scalar.copy(out=UTc[d][:, fsl], in_=ps1)
                        ps2 = ps_mm.tile([128, HALF], F32, tag="proj_ps")
                        for hh in range(HH):
                            nc.tensor.matmul(ps2, wk_eD[hh][:, dsl], T2T[hh][:, fsl],
                                             start=(hh == 0), stop=(hh == HH - 1))
                        nc.vector.tensor_add(UTc[d][:, fsl], UTc[d][:, fsl], ps2)

                # ---- Hopfield ----
                for hf in range(2):
                    fsl = slice(hf * HALF, (hf + 1) * HALF)
                    hids = []
                    for m in range(MC):
                        ps = ps_mm.tile([128, HALF], F32, tag="proj_ps")
                        for d in range(DC):
                            xb = xip.tile([128, 128], BF, tag="xib")
                            nc.sync.dma_start(out=xb, in_=d_xiblk[d, m, :, :])
                            nc.tensor.matmul(ps, xb, gT[d][:, fsl],
                                             start=(d == 0), stop=(d == DC - 1))
                        ht = hidp.tile([128, HALF], BF, tag=f"hid{m}", name=f"hid{m}")
                        nc.scalar.activation(out=ht, in_=ps, func=AF.Relu)
                        hids.append(ht)
                    for d in range(DC):
                        dsl = slice(d * 128, (d + 1) * 128)
                        ps = ps_mm.tile([128, HALF], F32, tag="proj_ps")
                        for m in range(MC):
                            nc.tensor.matmul(ps, xi_mD[m][:, dsl], hids[m],
                                             start=(m == 0), stop=(m == MC - 1))
                        nc.vector.tensor_add(UTc[d][:, fsl], UTc[d][:, fsl], ps)

                # ---- update: out += transpose(UTc) ----
                for d in range(DC):
                    dsl = slice(d * 128, (d + 1) * 128)
                    for s in range(SPC):
                        for c in range(2):
                            qsz = QSZ[c]
                            col = s * NT + c * 128
                            pst = ps_t.tile([128, 128], BF, tag="pst")
                            nc.tensor.transpose(pst[:qsz, :],
                                                UTc[d][:, col:col + qsz], ident)
                            ot = out_t[2 * s + c]
                            nc.vector.tensor_add(ot[:qsz, dsl], ot[:qsz, dsl],
                                                 pst[:qsz, :])

            if use_loop and n_iter > 1:
                with tc.For_i(0, n_iter, 1,
                              hint_engines=(mybir.EngineType.PE,
                                            mybir.EngineType.Activation,
                                            mybir.EngineType.DVE)) as _:
                    iteration_body()
            else:
                for _ in range(n_iter):
                    iteration_body()

        # ================= HEAD =================
        with tc.tile_pool(name="head", bufs=1) as hp, \
             tc.tile_pool(name="head_ps", bufs=2, space="PSUM") as hps:
            fcT_sb = ptiles(hp, DC, [128, NCLS], BF, "fcT")
            for i in range(DC):
                nc.sync.dma_start(out=fcT_sb[i], in_=d_fcwT[i * 128:(i + 1) * 128, :])
            lnw_sb = hp.tile([SPC, D], F32, tag="lnw")
            lnb_sb = hp.tile([SPC, D], F32, tag="lnb")
            fcb_sb = hp.tile([SPC, NCLS], F32, tag="fcb")
            nc.sync.dma_start(out=lnw_sb, in_=d_lnw[:, :])
            nc.sync.dma_start(out=lnb_sb, in_=d_lnb[:, :])
            nc.sync.dma_start(out=fcb_sb, in_=d_fcb[:, :])

            cls = hp.tile([SPC, D], F32, tag="cls")
            for s in range(SPC):
                nc.sync.dma_start(out=cls[s:s + 1, :],
                                  in_=out_t[2 * s + 1][CLS_ROW:CLS_ROW + 1, :])
            stats = hp.tile([SPC, 3, 6], F32, tag="hstats")
            mv = hp.tile([SPC, 2], F32, tag="hmv")
            for sg in range(3):
                nc.vector.bn_stats(out=stats[:, sg, :],
                                   in_=cls[:, sg * 256:(sg + 1) * 256])
            nc.vector.bn_aggr(out=mv, in_=stats)
            rstd = hp.tile([SPC, 1], F32, tag="hrstd")
            nc.scalar.activation(out=rstd, in_=mv[:, 1:2], func=AF.Sqrt,
                                 bias=eps_t[:SPC], scale=1.0)
            nc.vector.reciprocal(rstd, rstd)
            negmu = hp.tile([SPC, 1], F32, tag="hnegmu")
            nc.vector.tensor_mul(negmu, mv[:, 0:1], rstd)
            nc.vector.tensor_scalar_mul(negmu, negmu, -1.0)
            nhat = hp.tile([SPC, D], F32, tag="nhat")
            nc.scalar.activation(out=nhat, in_=cls, func=AF.Identity,
                                 bias=negmu, scale=rstd)
            nc.vector.tensor_mul(nhat, nhat, lnw_sb)
            chat = hp.tile([SPC, D], BF, tag="chat")
            nc.vector.tensor_add(chat, nhat, lnb_sb)
            cT = hp.tile([128, DC, SPC], BF, tag="cT")
            for d in range(DC):
                pst = hps.tile([128, SPC], BF, tag="hpst")
                nc.tensor.transpose(pst[:, :SPC], chat[:SPC, d * 128:(d + 1) * 128],
                                    ident[:SPC, :SPC])
                nc.scalar.copy(out=cT[:, d, :], in_=pst[:, :SPC])
            logits = hp.tile([SPC, NCLS], F32, tag="logits")
            for nh in range(2):
                nsl = slice(nh * 500, (nh + 1) * 500)
                psL = hps.tile([SPC, 500], F32, tag="psL")
                for d in range(DC):
                    nc.tensor.matmul(psL, cT[:, d, :], fcT_sb[d][:, nsl],
                                     start=(d == 0), stop=(d == DC - 1))
                nc.vector.tensor_add(logits[:, nsl], psL, fcb_sb[:, nsl])
            nc.sync.dma_start(out=d_y[:, :], in_=logits)

    nc.compile()
    return nc


def _patchify(x):
    b = x.shape[0]
    xx = x.reshape(b, 3, PATCH, PS, PATCH, PS)
    xx = xx.transpose(0, 2, 4, 3, 5, 1)  # b, ph, pw, ps1, ps2, c
    return xx.reshape(b, PATCH * PATCH, F)


def _bf(a):
    return np.ascontiguousarray(np.asarray(a, np.float32)).astype(ml_dtypes.bfloat16)


def _f32(a):
    return np.ascontiguousarray(np.asarray(a, np.float32))


def make_in_maps(x, emb_w, emb_b, cls_token, pos_emb, gamma, delta, wq, wk, xi,
                 ln_w, ln_b, fc_w, fc_b):
    x = np.asarray(x, np.float32)
    pos_emb = np.asarray(pos_emb, np.float32)
    emb_b = np.asarray(emb_b, np.float32)
    cls_token = np.asarray(cls_token, np.float32)
    wq = np.asarray(wq, np.float32)
    wk = np.asarray(wk, np.float32)
    xi = np.asarray(xi, np.float32)
    # token order: [patch 0..195, cls]
    posc = np.empty((NT, D), np.float32)
    posc[:NPATCH] = pos_emb[0, 1:] + np.asarray(emb_b, np.float32)[None, :]
    posc[NPATCH] = pos_emb[0, 0] + cls_token[0, 0]
    shared = {
        "embwT": _bf(np.asarray(emb_w, np.float32).T),
        "posc": _f32(posc),
        "wq_De": _bf(wq.transpose(1, 0, 2).reshape(D, D)),
        "wk_De": _bf(wk.transpose(1, 0, 2).reshape(D, D)),
        "wq_eD": _bf(wq.transpose(0, 2, 1).reshape(D, D)),
        "wk_eD": _bf(wk.transpose(0, 2, 1).reshape(D, D)),
        "xiblk": _bf(xi.reshape(DC, 128, MC, 128).transpose(0, 2, 1, 3)),
        "ximD": _bf(xi.T),
        "delta_bc": _bf(np.tile(np.asarray(delta, np.float32)[None, :], (128, 1))),
        "gamma_bc": _f32(np.full((128, 1), np.float32(gamma))),
        "ident_bf": _bf(np.eye(128)),
        "ones_bf": _bf(np.ones((1, 128))),
        "lnw_bc": _f32(np.tile(np.asarray(ln_w, np.float32)[None, :], (SPC, 1))),
        "lnb_bc": _f32(np.tile(np.asarray(ln_b, np.float32)[None, :], (SPC, 1))),
        "fcwT": _bf(np.asarray(fc_w, np.float32).T),
        "fcb_bc": _f32(np.tile(np.asarray(fc_b, np.float32)[None, :], (SPC, 1))),
    }
    in_maps = []
    for c in range(N_CORES):
        xs = x[c * SPC:(c + 1) * SPC]
        patches = _patchify(xs).reshape(SPC * NPATCH, F)
        m = dict(shared)
        m["pT"] = _bf(patches.T)
        in_maps.append(m)
    return in_maps


_NC_CACHE = None


def kernel(x, emb_w, emb_b, cls_token, pos_emb, gamma, delta, wq, wk, xi,
           ln_w, ln_b, fc_w, fc_b):
    global _NC_CACHE
    in_maps = make_in_maps(x, emb_w, emb_b, cls_token, pos_emb, gamma, delta,
                           wq, wk, xi, ln_w, ln_b, fc_w, fc_b)
    if _NC_CACHE is None:
        _NC_CACHE = build_nc()
    res = run_bass_kernel_spmd(_NC_CACHE, in_maps, core_ids=list(range(N_CORES)))
    return np.concatenate([r["y"] for r in res.results], axis=0)


# revision 16
# speedup vs baseline: 1.7353x; 1.7353x over previous
"""Energy Transformer (ET) Trainium2 kernel.

Data-parallel over batch: 32 samples -> 8 cores x 4 samples. Parameters
replicated; no collectives (inference only).

Math (verified against jax.grad to 6e-7):
  out += sum_h [ (P K) Wq_h^T + (P^T Q) Wk_h^T ] + relu(g xi) xi^T
  where g = energy-LN(out), P = softmax_k(beta Q K^T) per (sample, head).

Token order on device: [patch 0..195, cls] per sample (attention/LN are
permutation-invariant; pos-emb rows are pre-permuted on host). This keeps
the patch-embedding matmul partition-aligned with the residual tiles.

On-chip layouts per core (4 samples, 197 tokens each, TOK=788):
  out  : token-major fp32 residual, per-sample tiles (128|69, 768)
  gT   : feature-major bf16 (128, 6, 788) - matmul operand
  QT/KT: e-major bf16 (6 head-pair tiles x (128, 788))
  P^T is produced directly as exp(beta*S^T - lnZ) with the per-column lnZ
  injected via a rank-1 matmul accumulation (ones^T @ zrow).

v2: batched transpose-evacuations through multi-slot PSUM tiles, psum
evacs routed to DVE (ScalarE was the serialization bottleneck), lnZ row
construction batched per sample, xi streamed as 6-block bundles.
"""

import sys
for _p in ("/opt/trn_rl_repo",):
    if _p not in sys.path:
        sys.path.insert(0, _p)

import numpy as np
import ml_dtypes
from contextlib import ExitStack

import concourse.bass as bass
import concourse.bacc as bacc
import concourse.mybir as mybir
import concourse.tile as tile
from concourse.bass_utils import run_bass_kernel_spmd

BF = mybir.dt.bfloat16
F32 = mybir.dt.float32
AF = mybir.ActivationFunctionType

# ---- problem dims ----
N_CORES = 8
B = 32
SPC = B // N_CORES      # samples per core = 4
IMG = 224
PATCH = 14
PS = IMG // PATCH       # 16
F = PS * PS * 3         # 768
D = 768
DC = D // 128           # 6
H = 12
HH = H // 2             # 6 head-pairs
DK = 64
HN = 4 * D              # 3072
MC = HN // 128          # 24
NT = PATCH * PATCH + 1  # 197
NPATCH = NT - 1         # 196
TOK = SPC * NT          # 788
HALF = TOK // 2         # 394
NCLS = 1000
N_RECUR = 7
BETA = 1.0 / float(np.sqrt(DK))
EPS = 1e-5

QSZ = (128, NT - 128)      # per-sample token chunks: 128, 69
CLS_ROW = NPATCH - 128     # row 68 of chunk 1 holds the cls token

USE_LOOP = True            # tc.For_i over recurrence steps vs full unroll
N_ITER = N_RECUR


def build_nc(n_iter=N_ITER, use_loop=USE_LOOP):
    nc = bacc.Bacc(None, target_bir_lowering=False)

    # ---- DRAM I/O ----
    d_pT = nc.dram_tensor("pT", [F, SPC * NPATCH], BF, kind="ExternalInput")
    d_embwT = nc.dram_tensor("embwT", [F, D], BF, kind="ExternalInput")
    d_posc = nc.dram_tensor("posc", [NT, D], F32, kind="ExternalInput")
    d_wq_De = nc.dram_tensor("wq_De", [D, D], BF, kind="ExternalInput")
    d_wk_De = nc.dram_tensor("wk_De", [D, D], BF, kind="ExternalInput")
    d_wq_eD = nc.dram_tensor("wq_eD", [D, D], BF, kind="ExternalInput")
    d_wk_eD = nc.dram_tensor("wk_eD", [D, D], BF, kind="ExternalInput")
    # [m, p, d, f] = xi[d*128+p, m*128+f]: per-m bundle of 6 blocks
    d_xiblk = nc.dram_tensor("xiblk", [MC, 128, DC, 128], BF, kind="ExternalInput")
    d_ximD = nc.dram_tensor("ximD", [HN, D], BF, kind="ExternalInput")
    d_delta = nc.dram_tensor("delta_bc", [128, D], BF, kind="ExternalInput")
    d_gamma = nc.dram_tensor("gamma_bc", [128, 1], F32, kind="ExternalInput")
    d_ident = nc.dram_tensor("ident_bf", [128, 128], BF, kind="ExternalInput")
    d_ones = nc.dram_tensor("ones_bf", [1, 128], BF, kind="ExternalInput")
    d_lnw = nc.dram_tensor("lnw_bc", [SPC, D], F32, kind="ExternalInput")
    d_lnb = nc.dram_tensor("lnb_bc", [SPC, D], F32, kind="ExternalInput")
    d_fcwT = nc.dram_tensor("fcwT", [D, NCLS], BF, kind="ExternalInput")
    d_fcb = nc.dram_tensor("fcb_bc", [SPC, NCLS], F32, kind="ExternalInput")
    d_y = nc.dram_tensor("y", [SPC, NCLS], F32, kind="ExternalOutput")

    with ExitStack() as ctx:
        tc = ctx.enter_context(tile.TileContext(nc))
        consts = ctx.enter_context(tc.tile_pool(name="consts", bufs=1))
        state = ctx.enter_context(tc.tile_pool(name="state", bufs=1))

        def ptiles(pool, n, shape, dtype, tag):
            return [pool.tile(shape, dtype, tag=f"{tag}{i}", name=f"{tag}{i}")
                    for i in range(n)]

        # ---- resident constants ----
        wq_De = ptiles(consts, DC, [128, D], BF, "wq_De")
        wk_De = ptiles(consts, DC, [128, D], BF, "wk_De")
        wq_eD = ptiles(consts, HH, [128, D], BF, "wq_eD")
        wk_eD = ptiles(consts, HH, [128, D], BF, "wk_eD")
        xi_mD = ptiles(consts, MC, [128, D], BF, "xi_mD")
        delta_bc = consts.tile([128, D], BF, tag="delta_bc")
        gamma_bc = consts.tile([128, 1], F32, tag="gamma_bc")
        eps_t = consts.tile([128, 1], F32, tag="eps_t")
        ident = consts.tile([128, 128], BF, tag="ident")
        ones_r = consts.tile([1, 128], BF, tag="ones_r")

        for i in range(DC):
            nc.sync.dma_start(out=wq_De[i], in_=d_wq_De[i * 128:(i + 1) * 128, :])
            nc.sync.dma_start(out=wk_De[i], in_=d_wk_De[i * 128:(i + 1) * 128, :])
        for i in range(HH):
            nc.sync.dma_start(out=wq_eD[i], in_=d_wq_eD[i * 128:(i + 1) * 128, :])
            nc.sync.dma_start(out=wk_eD[i], in_=d_wk_eD[i * 128:(i + 1) * 128, :])
        for i in range(MC):
            nc.sync.dma_start(out=xi_mD[i], in_=d_ximD[i * 128:(i + 1) * 128, :])
        nc.sync.dma_start(out=delta_bc, in_=d_delta[:, :])
        nc.sync.dma_start(out=gamma_bc, in_=d_gamma[:, :])
        nc.sync.dma_start(out=ident, in_=d_ident[:, :])
        nc.sync.dma_start(out=ones_r, in_=d_ones[:, :])
        nc.vector.memset(eps_t, EPS)

        # ---- persistent state ----
        out_t = []  # [2*s + c]
        for s in range(SPC):
            for c in range(2):
                out_t.append(state.tile([QSZ[c], D], F32, tag=f"out_{s}_{c}",
                                        name=f"out_{s}_{c}"))
        gT = state.tile([128, DC, TOK], BF, tag="gT")   # [p, d, tok]
        QT = ptiles(state, HH, [128, TOK], BF, "QT")
        KT = ptiles(state, HH, [128, TOK], BF, "KT")
        U1T = ptiles(state, HH, [128, TOK], BF, "U1T")
        T2T = ptiles(state, HH, [128, TOK], BF, "T2T")
        UTc = ptiles(state, DC, [128, TOK], BF, "UTc")

        # ================= INIT: patch embedding =================
        with tc.tile_pool(name="init", bufs=1) as initp, \
             tc.tile_pool(name="init_ps", bufs=4, space="PSUM") as ipsum:
            pT_sb = ptiles(initp, DC, [128, SPC * NPATCH], BF, "pT")
            embT_sb = ptiles(initp, DC, [128, D], BF, "embT")
            posc_sb = [initp.tile([QSZ[c], D], F32, tag=f"posc{c}", name=f"posc{c}")
                       for c in range(2)]
            for i in range(DC):
                nc.sync.dma_start(out=pT_sb[i], in_=d_pT[i * 128:(i + 1) * 128, :])
                nc.sync.dma_start(out=embT_sb[i], in_=d_embwT[i * 128:(i + 1) * 128, :])
            nc.sync.dma_start(out=posc_sb[0], in_=d_posc[0:128, :])
            nc.sync.dma_start(out=posc_sb[1], in_=d_posc[128:NT, :])

            for s in range(SPC):
                for pc in range(2):
                    psz = (128, CLS_ROW)[pc]   # 128 / 68 patches
                    for nh in range(2):
                        nsl = slice(nh * 384, (nh + 1) * 384)
                        ps = ipsum.tile([128, 384], F32, tag="emb_ps")
                        for fi in range(DC):
                            nc.tensor.matmul(
                                ps[:psz],
                                pT_sb[fi][:, s * NPATCH + pc * 128:
                                          s * NPATCH + pc * 128 + psz],
                                embT_sb[fi][:, nsl],
                                start=(fi == 0), stop=(fi == DC - 1))
                        nc.vector.tensor_add(
                            out_t[2 * s + pc][0:psz, nsl], ps[0:psz, :],
                            posc_sb[pc][0:psz, nsl])
                # cls token row: chunk1 row CLS_ROW (no patch contribution)
                nc.sync.dma_start(out=out_t[2 * s + 1][CLS_ROW:CLS_ROW + 1, :],
                                  in_=posc_sb[1][CLS_ROW:CLS_ROW + 1, :])

        # ================= RECURRENCE =================
        with tc.tile_pool(name="lnp", bufs=2) as lnp, \
             tc.tile_pool(name="gtk", bufs=2) as gtk, \
             tc.tile_pool(name="atk", bufs=1) as atk, \
             tc.tile_pool(name="atn", bufs=4) as atn, \
             tc.tile_pool(name="zp", bufs=2) as zp, \
             tc.tile_pool(name="hidp", bufs=1) as hidp, \
             tc.tile_pool(name="xip", bufs=3) as xip, \
             tc.tile_pool(name="ps_t", bufs=2, space="PSUM") as ps_t, \
             tc.tile_pool(name="ps_mm", bufs=2, space="PSUM") as ps_mm, \
             tc.tile_pool(name="ps_at", bufs=2, space="PSUM") as ps_at, \
             tc.tile_pool(name="ps_sm", bufs=2, space="PSUM") as ps_sm:

            def iteration_body():
                # ---- LN + transpose g (batched evac) ----
                for s in range(SPC):
                    for c in range(2):
                        qsz = QSZ[c]
                        ot = out_t[2 * s + c]
                        stats = lnp.tile([128, 3, 6], F32, tag="bnstats")
                        mv = lnp.tile([128, 2], F32, tag="mv")
                        for sg in range(3):
                            nc.vector.bn_stats(out=stats[:qsz, sg, :],
                                               in_=ot[:qsz, sg * 256:(sg + 1) * 256])
                        nc.vector.bn_aggr(out=mv[:qsz], in_=stats[:qsz])
                        rstd = lnp.tile([128, 1], F32, tag="rstd")
                        nc.scalar.activation(out=rstd[:qsz], in_=mv[:qsz, 1:2],
                                             func=AF.Sqrt, bias=eps_t[:qsz], scale=1.0)
                        nc.vector.reciprocal(rstd[:qsz], rstd[:qsz])
                        nc.vector.tensor_mul(rstd[:qsz], rstd[:qsz], gamma_bc[:qsz])
                        negmu = lnp.tile([128, 1], F32, tag="negmu")
                        nc.vector.tensor_mul(negmu[:qsz], mv[:qsz, 0:1], rstd[:qsz])
                        nc.vector.tensor_scalar_mul(negmu[:qsz], negmu[:qsz], -1.0)
                        g1 = lnp.tile([128, D], F32, tag="g1")
                        nc.scalar.activation(out=g1[:qsz], in_=ot[:qsz],
                                             func=AF.Identity,
                                             bias=negmu[:qsz], scale=rstd[:qsz])
                        gtok = gtk.tile([128, D], BF, tag="gtok")
                        nc.vector.tensor_add(gtok[:qsz], g1[:qsz], delta_bc[:qsz])
                        col = s * NT + c * 128
                        pst = ps_t.tile([128, DC, 128], BF, tag="pst")
                        for d in range(DC):
                            nc.tensor.transpose(pst[:, d, :qsz],
                                                gtok[:qsz, d * 128:(d + 1) * 128],
                                                ident[:qsz, :qsz])
                        nc.vector.tensor_copy(out=gT[:, :, col:col + qsz],
                                              in_=pst[:, :, :qsz])

                # ---- Q/K projections (e-major) ----
                for (W, OUT) in ((wq_De, QT), (wk_De, KT)):
                    for hh in range(HH):
                        for hf in range(2):
                            fsl = slice(hf * HALF, (hf + 1) * HALF)
                            ps = ps_mm.tile([128, HALF], F32, tag="proj_ps")
                            for d in range(DC):
                                nc.tensor.matmul(
                                    ps, W[d][:, hh * 128:(hh + 1) * 128],
                                    gT[:, d, fsl],
                                    start=(d == 0), stop=(d == DC - 1))
                            nc.scalar.copy(out=OUT[hh][:, fsl], in_=ps)

                # ---- attention per sample ----
                for s in range(SPC):
                    scol = slice(s * NT, (s + 1) * NT)
                    # token-major Q/K for this sample (batched transp-evac)
                    Qs = [atk.tile([128, D], BF, tag=f"Qs{c}", name=f"Qs{c}")
                          for c in range(2)]
                    Ks = [atk.tile([128, D], BF, tag=f"Ks{c}", name=f"Ks{c}")
                          for c in range(2)]
                    for (SRC, DST) in ((QT, Qs), (KT, Ks)):
                        for c in range(2):
                            qsz = QSZ[c]
                            col = s * NT + c * 128
                            pst = ps_t.tile([128, DC, 128], BF, tag="pst")
                            for hh in range(HH):
                                nc.tensor.transpose(pst[:qsz, hh, :],
                                                    SRC[hh][:, col:col + qsz], ident)
                            nc.vector.tensor_copy(
                                out=DST[c][:qsz, :].rearrange(
                                    "p (a b) -> p a b", a=DC),
                                in_=pst[:qsz, :, :])
                    # pass A: per head E-side (S, exp+Z, P, T2^T)
                    Z_all = [zp.tile([128, H], F32, tag=f"Z_all{c}", name=f"Z_all{c}")
                             for c in range(2)]
                    Es = {}
                    for h in range(H):
                        hh, ho = h // 2, (h % 2) * 64
                        esl = slice(ho, ho + 64)
                        hsl = slice(h * 64, (h + 1) * 64)
                        for c in range(2):
                            qsz = QSZ[c]
                            col = s * NT + c * 128
                            ps = ps_at.tile([128, NT], F32, tag="psS")
                            nc.tensor.matmul(ps[:qsz], QT[hh][esl, col:col + qsz],
                                             KT[hh][esl, scol], start=True, stop=True)
                            E = atn.tile([128, NT], BF, tag="E")
                            nc.scalar.activation(out=E[:qsz], in_=ps[:qsz],
                                                 func=AF.Exp, scale=BETA,
                                                 accum_out=Z_all[c][:qsz, h:h + 1])
                            Zi = atn.tile([128, 1], F32, tag="Zi")
                            nc.vector.reciprocal(Zi[:qsz], Z_all[c][:qsz, h:h + 1])
                            nc.vector.tensor_scalar_mul(E[:qsz], E[:qsz], Zi[:qsz])
                            Es[c] = E
                        psT2 = ps_sm.tile([64, NT], F32, tag="ps_sm")
                        for c in range(2):
                            qsz = QSZ[c]
                            nc.tensor.matmul(psT2, Qs[c][:qsz, hsl], Es[c][:qsz],
                                             start=(c == 0), stop=(c == 1))
                        nc.vector.tensor_copy(out=T2T[hh][esl, scol], in_=psT2)
                    # batched -lnZ/beta rows: (qsz,12) -> T -> (12,197) -> flat
                    zrS = zp.tile([H, NT], BF, tag="zrS")
                    zps = ps_t.tile([H, NT], BF, tag="pst")
                    for c in range(2):
                        qsz = QSZ[c]
                        nl = zp.tile([128, H], BF, tag=f"nl{c}", name=f"nl{c}")
                        nc.scalar.activation(out=nl[:qsz], in_=Z_all[c][:qsz],
                                             func=AF.Ln, scale=1.0)
                        nc.vector.tensor_scalar_mul(nl[:qsz], nl[:qsz], -1.0 / BETA)
                        nc.tensor.transpose(zps[:, c * 128:c * 128 + qsz],
                                            nl[:qsz, :], ident[:qsz, :qsz])
                    nc.vector.tensor_copy(out=zrS, in_=zps)
                    zrF = zp.tile([1, H, NT], BF, tag="zrF")
                    nc.sync.dma_start(out=zrF, in_=zrS)
                    # pass B: per head ST-side (S^T + lnZ bias, exp -> P^T, U1)
                    PTs = {}
                    for h in range(H):
                        hh, ho = h // 2, (h % 2) * 64
                        esl = slice(ho, ho + 64)
                        hsl = slice(h * 64, (h + 1) * 64)
                        for c in range(2):
                            ksz = QSZ[c]
                            col = s * NT + c * 128
                            ps = ps_at.tile([128, NT], F32, tag="psS")
                            nc.tensor.matmul(ps[:ksz], KT[hh][esl, col:col + ksz],
                                             QT[hh][esl, scol], start=True, stop=False)
                            nc.tensor.matmul(ps[:ksz], ones_r[:1, :ksz],
                                             zrF[:, h, :], start=False, stop=True)
                            PT = atn.tile([128, NT], BF, tag="PT")
                            nc.scalar.activation(out=PT[:ksz], in_=ps[:ksz],
                                                 func=AF.Exp, scale=BETA)
                            PTs[c] = PT
                        psU = ps_sm.tile([64, NT], F32, tag="ps_sm")
                        for c in range(2):
                            ksz = QSZ[c]
                            nc.tensor.matmul(psU, Ks[c][:ksz, hsl], PTs[c][:ksz],
                                             start=(c == 0), stop=(c == 1))
                        nc.vector.tensor_copy(out=U1T[hh][esl, scol], in_=psU)

                # ---- attention output projections ----
                for d in range(DC):
                    dsl = slice(d * 128, (d + 1) * 128)
                    for hf in range(2):
                        fsl = slice(hf * HALF, (hf + 1) * HALF)
                        ps1 = ps_mm.tile([128, HALF], F32, tag="proj_ps")
                        for hh in range(HH):
                            nc.tensor.matmul(ps1, wq_eD[hh][:, dsl], U1T[hh][:, fsl],
                                             start=(hh == 0), stop=(hh == HH - 1))
                        nc.vector.tensor_copy(out=UTc[d][:, fsl], in_=ps1)
                        ps2 = ps_mm.tile([128, HALF], F32, tag="proj_ps")
                        for hh in range(HH):
                            nc.tensor.matmul(ps2, wk_eD[hh][:, dsl], T2T[hh][:, fsl],
                                             start=(hh == 0), stop=(hh == HH - 1))
                        nc.vector.tensor_add(UTc[d][:, fsl], UTc[d][:, fsl], ps2)

                # ---- Hopfield ----
                for hf in range(2):
                    fsl = slice(hf * HALF, (hf + 1) * HALF)
                    hids = []
                    for m in range(MC):
                        xb = xip.tile([128, DC, 128], BF, tag="xib")
                        nc.sync.dma_start(out=xb, in_=d_xiblk[m])
                        ps = ps_mm.tile([128, HALF], F32, tag="proj_ps")
                        for d in range(DC):
                            nc.tensor.matmul(ps, xb[:, d, :], gT[:, d, fsl],
                                             start=(d == 0), stop=(d == DC - 1))
                        ht = hidp.tile([128, HALF], BF, tag=f"hid{m}", name=f"hid{m}")
                        nc.scalar.activation(out=ht, in_=ps, func=AF.Relu)
                        hids.append(ht)
                    for d in range(DC):
                        dsl = slice(d * 128, (d + 1) * 128)
                        ps = ps_mm.tile([128, HALF], F32, tag="proj_ps")
                        for m in range(MC):
                            nc.tensor.matmul(ps, xi_mD[m][:, dsl], hids[m],
                                             start=(m == 0), stop=(m == MC - 1))
                        nc.vector.tensor_add(UTc[d][:, fsl], UTc[d][:, fsl], ps)

                # ---- update: out += transpose(UTc), batched per (s,c) ----
                for s in range(SPC):
                    for c in range(2):
                        qsz = QSZ[c]
                        col = s * NT + c * 128
                        pst = ps_t.tile([128, DC, 128], BF, tag="pst")
                        for d in range(DC):
                            nc.tensor.transpose(pst[:qsz, d, :],
                                                UTc[d][:, col:col + qsz], ident)
                        ot = out_t[2 * s + c]
                        nc.vector.tensor_add(
                            ot[:qsz, :], ot[:qsz, :],
                            pst[:qsz, :, :].rearrange("p a b -> p (a b)"))

            if use_loop and n_iter > 1:
                with tc.For_i(0, n_iter, 1,
                              hint_engines=(mybir.EngineType.PE,
                                            mybir.EngineType.Activation,
                                            mybir.EngineType.DVE)) as _:
                    iteration_body()
            else:
                for _ in range(n_iter):
                    iteration_body()

        # ================= HEAD =================
        with tc.tile_pool(name="head", bufs=1) as hp, \
             tc.tile_pool(name="head_ps", bufs=2, space="PSUM") as hps:
            fcT_sb = ptiles(hp, DC, [128, NCLS], BF, "fcT")
            for i in range(DC):
                nc.sync.dma_start(out=fcT_sb[i], in_=d_fcwT[i * 128:(i + 1) * 128, :])
            lnw_sb = hp.tile([SPC, D], F32, tag="lnw")
            lnb_sb = hp.tile([SPC, D], F32, tag="lnb")
            fcb_sb = hp.tile([SPC, NCLS], F32, tag="fcb")
            nc.sync.dma_start(out=lnw_sb, in_=d_lnw[:, :])
            nc.sync.dma_start(out=lnb_sb, in_=d_lnb[:, :])
            nc.sync.dma_start(out=fcb_sb, in_=d_fcb[:, :])

            cls = hp.tile([SPC, D], F32, tag="cls")
            for s in range(SPC):
                nc.sync.dma_start(out=cls[s:s + 1, :],
                                  in_=out_t[2 * s + 1][CLS_ROW:CLS_ROW + 1, :])
            stats = hp.tile([SPC, 3, 6], F32, tag="hstats")
            mv = hp.tile([SPC, 2], F32, tag="hmv")
            for sg in range(3):
                nc.vector.bn_stats(out=stats[:, sg, :],
                                   in_=cls[:, sg * 256:(sg + 1) * 256])
            nc.vector.bn_aggr(out=mv, in_=stats)
            rstd = hp.tile([SPC, 1], F32, tag="hrstd")
            nc.scalar.activation(out=rstd, in_=mv[:, 1:2], func=AF.Sqrt,
                                 bias=eps_t[:SPC], scale=1.0)
            nc.vector.reciprocal(rstd, rstd)
            negmu = hp.tile([SPC, 1], F32, tag="hnegmu")
            nc.vector.tensor_mul(negmu, mv[:, 0:1], rstd)
            nc.vector.tensor_scalar_mul(negmu, negmu, -1.0)
            nhat = hp.tile([SPC, D], F32, tag="nhat")
            nc.scalar.activation(out=nhat, in_=cls, func=AF.Identity,
                                 bias=negmu, scale=rstd)
            nc.vector.tensor_mul(nhat, nhat, lnw_sb)
            chat = hp.tile([SPC, D], BF, tag="chat")
            nc.vector.tensor_add(chat, nhat, lnb_sb)
            cT = hp.tile([128, DC, SPC], BF, tag="cT")
            for d in range(DC):
                pst = hps.tile([128, SPC], BF, tag="hpst")
                nc.tensor.transpose(pst[:, :SPC], chat[:SPC, d * 128:(d + 1) * 128],
                                    ident[:SPC, :SPC])
                nc.scalar.copy(out=cT[:, d, :], in_=pst[:, :SPC])
            logits = hp.tile([SPC, NCLS], F32, tag="logits")
            for nh in range(2):
                nsl = slice(nh * 500, (nh + 1) * 500)
                psL = hps.tile([SPC, 500], F32, tag="psL")
                for d in range(DC):
                    nc.tensor.matmul(psL, cT[:, d, :], fcT_sb[d][:, nsl],
                                     start=(d == 0), stop=(d == DC - 1))
                nc.vector.tensor_add(logits[:, nsl], psL, fcb_sb[:, nsl])
            nc.sync.dma_start(out=d_y[:, :], in_=logits)

    nc.compile()
    return nc


def _patchify(x):
    b = x.shape[0]
    xx = x.reshape(b, 3, PATCH, PS, PATCH, PS)
    xx = xx.transpose(0, 2, 4, 3, 5, 1)  # b, ph, pw, ps1, ps2, c
    return xx.reshape(b, PATCH * PATCH, F)


def _bf(a):
    return np.ascontiguousarray(np.asarray(a, np.float32)).astype(ml_dtypes.bfloat16)


def _f32(a):
    return np.ascontiguousarray(np.asarray(a, np.float32))


def make_in_maps(x, emb_w, emb_b, cls_token, pos_emb, gamma, delta, wq, wk, xi,
                 ln_w, ln_b, fc_w, fc_b):
    x = np.asarray(x, np.float32)
    pos_emb = np.asarray(pos_emb, np.float32)
    emb_b = np.asarray(emb_b, np.float32)
    cls_token = np.asarray(cls_token, np.float32)
    wq = np.asarray(wq, np.float32)
    wk = np.asarray(wk, np.float32)
    xi = np.asarray(xi, np.float32)
    # token order: [patch 0..195, cls]
    posc = np.empty((NT, D), np.float32)
    posc[:NPATCH] = pos_emb[0, 1:] + np.asarray(emb_b, np.float32)[None, :]
    posc[NPATCH] = pos_emb[0, 0] + cls_token[0, 0]
    shared = {
        "embwT": _bf(np.asarray(emb_w, np.float32).T),
        "posc": _f32(posc),
        "wq_De": _bf(wq.transpose(1, 0, 2).reshape(D, D)),
        "wk_De": _bf(wk.transpose(1, 0, 2).reshape(D, D)),
        "wq_eD": _bf(wq.transpose(0, 2, 1).reshape(D, D)),
        "wk_eD": _bf(wk.transpose(0, 2, 1).reshape(D, D)),
        # [m, p, d, f] = xi[d*128+p, m*128+f]
        "xiblk": _bf(xi.reshape(DC, 128, MC, 128).transpose(2, 1, 0, 3)),
        "ximD": _bf(xi.T),
        "delta_bc": _bf(np.tile(np.asarray(delta, np.float32)[None, :], (128, 1))),
        "gamma_bc": _f32(np.full((128, 1), np.float32(gamma))),
        "ident_bf": _bf(np.eye(128)),
        "ones_bf": _bf(np.ones((1, 128))),
        "lnw_bc": _f32(np.tile(np.asarray(ln_w, np.float32)[None, :], (SPC, 1))),
        "lnb_bc": _f32(np.tile(np.asarray(ln_b, np.float32)[None, :], (SPC, 1))),
        "fcwT": _bf(np.asarray(fc_w, np.float32).T),
        "fcb_bc": _f32(np.tile(np.asarray(fc_b, np.float32)[None, :], (SPC, 1))),
    }
    in_maps = []
    for c in range(N_CORES):
        xs = x[c * SPC:(c + 1) * SPC]
        patches = _patchify(xs).reshape(SPC * NPATCH, F)
        m = dict(shared)
        m["pT"] = _bf(patches.T)
        in_maps.append(m)
    return in_maps


_NC_CACHE = None


def kernel(x, emb_w, emb_b, cls_token, pos_emb, gamma, delta, wq, wk, xi,
           ln_w, ln_b, fc_w, fc_b):
    global _NC_CACHE
    in_maps = make_in_maps(x, emb_w, emb_b, cls_token, pos_emb, gamma, delta,
                           wq, wk, xi, ln_w, ln_b, fc_w, fc_b)
    if _NC_CACHE is None:
        _NC_CACHE = build_nc()
    res = run_bass_kernel_spmd(_NC_CACHE, in_maps, core_ids=list(range(N_CORES)))
    return np.concatenate([r["y"] for r in res.results], axis=0)


# revision 18
# speedup vs baseline: 2.0123x; 1.1596x over previous
"""Energy Transformer (ET) Trainium2 kernel.

Data-parallel over batch: 32 samples -> 8 cores x 4 samples. Parameters
replicated; no collectives (inference only).

Math (verified against jax.grad to 6e-7):
  out += sum_h [ (P K) Wq_h^T + (P^T Q) Wk_h^T ] + relu(g xi) xi^T
  where g = energy-LN(out), P = softmax_k(beta Q K^T) per (sample, head).

Token order on device: [patch 0..195, cls] per sample (attention/LN are
permutation-invariant; pos-emb rows are pre-permuted on host). This keeps
the patch-embedding matmul partition-aligned with the residual tiles.

On-chip layouts per core (4 samples, 197 tokens each, TOK=788):
  out  : token-major fp32 residual, per-sample tiles (128|69, 768)
  gT   : feature-major bf16 (128, 6, 788) - matmul operand
  QT/KT: e-major bf16 (6 head-pair tiles x (128, 788))
  P^T is produced directly as exp(beta*S^T - lnZ) with the per-column lnZ
  injected via a rank-1 matmul accumulation (ones^T @ zrow).

v2: batched transpose-evacuations through multi-slot PSUM tiles, psum
evacs routed to DVE (ScalarE was the serialization bottleneck), lnZ row
construction batched per sample, xi streamed as 6-block bundles.
"""

import sys
for _p in ("/opt/trn_rl_repo",):
    if _p not in sys.path:
        sys.path.insert(0, _p)

import numpy as np
import ml_dtypes
from contextlib import ExitStack

import concourse.bass as bass
import concourse.bacc as bacc
import concourse.mybir as mybir
import concourse.tile as tile
from concourse.bass_utils import run_bass_kernel_spmd

BF = mybir.dt.bfloat16
F32 = mybir.dt.float32
AF = mybir.ActivationFunctionType

# ---- problem dims ----
N_CORES = 8
B = 32
SPC = B // N_CORES      # samples per core = 4
IMG = 224
PATCH = 14
PS = IMG // PATCH       # 16
F = PS * PS * 3         # 768
D = 768
DC = D // 128           # 6
H = 12
HH = H // 2             # 6 head-pairs
DK = 64
HN = 4 * D              # 3072
MC = HN // 128          # 24
NT = PATCH * PATCH + 1  # 197
NPATCH = NT - 1         # 196
TOK = SPC * NT          # 788
HALF = TOK // 2         # 394
NCLS = 1000
N_RECUR = 7
BETA = 1.0 / float(np.sqrt(DK))
EPS = 1e-5

QSZ = (128, NT - 128)      # per-sample token chunks: 128, 69
CLS_ROW = NPATCH - 128     # row 68 of chunk 1 holds the cls token

USE_LOOP = True            # tc.For_i over recurrence steps vs full unroll
N_ITER = N_RECUR


def build_nc(n_iter=N_ITER, use_loop=USE_LOOP):
    nc = bacc.Bacc(None, target_bir_lowering=False)

    # ---- DRAM I/O ----
    d_pT = nc.dram_tensor("pT", [F, SPC * NPATCH], BF, kind="ExternalInput")
    d_embwT = nc.dram_tensor("embwT", [F, D], BF, kind="ExternalInput")
    d_posc = nc.dram_tensor("posc", [NT, D], F32, kind="ExternalInput")
    d_wq_De = nc.dram_tensor("wq_De", [D, D], BF, kind="ExternalInput")
    d_wk_De = nc.dram_tensor("wk_De", [D, D], BF, kind="ExternalInput")
    d_wq_eD = nc.dram_tensor("wq_eD", [D, D], BF, kind="ExternalInput")
    d_wk_eD = nc.dram_tensor("wk_eD", [D, D], BF, kind="ExternalInput")
    # [m, p, d, f] = xi[d*128+p, m*128+f]: per-m bundle of 6 blocks
    d_xiblk = nc.dram_tensor("xiblk", [MC, 128, DC, 128], BF, kind="ExternalInput")
    d_ximD = nc.dram_tensor("ximD", [HN, D], BF, kind="ExternalInput")
    d_delta = nc.dram_tensor("delta_bc", [128, D], BF, kind="ExternalInput")
    d_gamma = nc.dram_tensor("gamma_bc", [128, 1], F32, kind="ExternalInput")
    d_ident = nc.dram_tensor("ident_bf", [128, 128], BF, kind="ExternalInput")
    d_ones = nc.dram_tensor("ones_bf", [1, 128], BF, kind="ExternalInput")
    d_lnw = nc.dram_tensor("lnw_bc", [SPC, D], F32, kind="ExternalInput")
    d_lnb = nc.dram_tensor("lnb_bc", [SPC, D], F32, kind="ExternalInput")
    d_fcwT = nc.dram_tensor("fcwT", [D, NCLS], BF, kind="ExternalInput")
    d_fcb = nc.dram_tensor("fcb_bc", [SPC, NCLS], F32, kind="ExternalInput")
    d_y = nc.dram_tensor("y", [SPC, NCLS], F32, kind="ExternalOutput")

    with ExitStack() as ctx:
        tc = ctx.enter_context(tile.TileContext(nc))
        consts = ctx.enter_context(tc.tile_pool(name="consts", bufs=1))
        state = ctx.enter_context(tc.tile_pool(name="state", bufs=1))

        def ptiles(pool, n, shape, dtype, tag):
            return [pool.tile(shape, dtype, tag=f"{tag}{i}", name=f"{tag}{i}")
                    for i in range(n)]

        # ---- resident constants ----
        wq_De = ptiles(consts, DC, [128, D], BF, "wq_De")
        wk_De = ptiles(consts, DC, [128, D], BF, "wk_De")
        wq_eD = ptiles(consts, HH, [128, D], BF, "wq_eD")
        wk_eD = ptiles(consts, HH, [128, D], BF, "wk_eD")
        xi_mD = ptiles(consts, MC, [128, D], BF, "xi_mD")
        delta_bc = consts.tile([128, D], BF, tag="delta_bc")
        gamma_bc = consts.tile([128, 1], F32, tag="gamma_bc")
        eps_t = consts.tile([128, 1], F32, tag="eps_t")
        ident = consts.tile([128, 128], BF, tag="ident")
        ones_r = consts.tile([1, 128], BF, tag="ones_r")

        for i in range(DC):
            nc.sync.dma_start(out=wq_De[i], in_=d_wq_De[i * 128:(i + 1) * 128, :])
            nc.sync.dma_start(out=wk_De[i], in_=d_wk_De[i * 128:(i + 1) * 128, :])
        for i in range(HH):
            nc.sync.dma_start(out=wq_eD[i], in_=d_wq_eD[i * 128:(i + 1) * 128, :])
            nc.sync.dma_start(out=wk_eD[i], in_=d_wk_eD[i * 128:(i + 1) * 128, :])
        for i in range(MC):
            nc.sync.dma_start(out=xi_mD[i], in_=d_ximD[i * 128:(i + 1) * 128, :])
        nc.sync.dma_start(out=delta_bc, in_=d_delta[:, :])
        nc.sync.dma_start(out=gamma_bc, in_=d_gamma[:, :])
        nc.sync.dma_start(out=ident, in_=d_ident[:, :])
        nc.sync.dma_start(out=ones_r, in_=d_ones[:, :])
        nc.vector.memset(eps_t, EPS)

        # ---- persistent state ----
        out_t = []  # [2*s + c]
        for s in range(SPC):
            for c in range(2):
                out_t.append(state.tile([QSZ[c], D], F32, tag=f"out_{s}_{c}",
                                        name=f"out_{s}_{c}"))
        gT = state.tile([128, DC, TOK], BF, tag="gT")   # [p, d, tok]
        QT = ptiles(state, HH, [128, TOK], BF, "QT")
        KT = ptiles(state, HH, [128, TOK], BF, "KT")
        U1T = ptiles(state, HH, [128, TOK], BF, "U1T")
        T2T = ptiles(state, HH, [128, TOK], BF, "T2T")
        UTc = ptiles(state, DC, [128, TOK], BF, "UTc")

        # ================= INIT: patch embedding =================
        with tc.tile_pool(name="init", bufs=1) as initp, \
             tc.tile_pool(name="init_ps", bufs=4, space="PSUM") as ipsum:
            pT_sb = ptiles(initp, DC, [128, SPC * NPATCH], BF, "pT")
            embT_sb = ptiles(initp, DC, [128, D], BF, "embT")
            posc_sb = [initp.tile([QSZ[c], D], F32, tag=f"posc{c}", name=f"posc{c}")
                       for c in range(2)]
            for i in range(DC):
                nc.sync.dma_start(out=pT_sb[i], in_=d_pT[i * 128:(i + 1) * 128, :])
                nc.sync.dma_start(out=embT_sb[i], in_=d_embwT[i * 128:(i + 1) * 128, :])
            nc.sync.dma_start(out=posc_sb[0], in_=d_posc[0:128, :])
            nc.sync.dma_start(out=posc_sb[1], in_=d_posc[128:NT, :])

            for s in range(SPC):
                for pc in range(2):
                    psz = (128, CLS_ROW)[pc]   # 128 / 68 patches
                    for nh in range(2):
                        nsl = slice(nh * 384, (nh + 1) * 384)
                        ps = ipsum.tile([128, 384], F32, tag="emb_ps")
                        for fi in range(DC):
                            nc.tensor.matmul(
                                ps[:psz],
                                pT_sb[fi][:, s * NPATCH + pc * 128:
                                          s * NPATCH + pc * 128 + psz],
                                embT_sb[fi][:, nsl],
                                start=(fi == 0), stop=(fi == DC - 1))
                        nc.vector.tensor_add(
                            out_t[2 * s + pc][0:psz, nsl], ps[0:psz, :],
                            posc_sb[pc][0:psz, nsl])
                # cls token row: chunk1 row CLS_ROW (no patch contribution)
                nc.sync.dma_start(out=out_t[2 * s + 1][CLS_ROW:CLS_ROW + 1, :],
                                  in_=posc_sb[1][CLS_ROW:CLS_ROW + 1, :])

        # ================= RECURRENCE =================
        with tc.tile_pool(name="lnp", bufs=2) as lnp, \
             tc.tile_pool(name="gtk", bufs=2) as gtk, \
             tc.tile_pool(name="atk", bufs=1) as atk, \
             tc.tile_pool(name="atn", bufs=4) as atn, \
             tc.tile_pool(name="hidp", bufs=1) as hidp, \
             tc.tile_pool(name="xip", bufs=3) as xip, \
             tc.tile_pool(name="ps_t", bufs=2, space="PSUM") as ps_t, \
             tc.tile_pool(name="ps_mm", bufs=2, space="PSUM") as ps_mm, \
             tc.tile_pool(name="ps_at", bufs=2, space="PSUM") as ps_at, \
             tc.tile_pool(name="ps_sm", bufs=2, space="PSUM") as ps_sm:

            def iteration_body():
                # ---- LN + transpose g (batched evac) ----
                for s in range(SPC):
                    for c in range(2):
                        qsz = QSZ[c]
                        ot = out_t[2 * s + c]
                        stats = lnp.tile([128, 3, 6], F32, tag="bnstats")
                        mv = lnp.tile([128, 2], F32, tag="mv")
                        for sg in range(3):
                            nc.vector.bn_stats(out=stats[:qsz, sg, :],
                                               in_=ot[:qsz, sg * 256:(sg + 1) * 256])
                        nc.vector.bn_aggr(out=mv[:qsz], in_=stats[:qsz])
                        rstd = lnp.tile([128, 1], F32, tag="rstd")
                        nc.scalar.activation(out=rstd[:qsz], in_=mv[:qsz, 1:2],
                                             func=AF.Sqrt, bias=eps_t[:qsz], scale=1.0)
                        nc.vector.reciprocal(rstd[:qsz], rstd[:qsz])
                        nc.vector.tensor_mul(rstd[:qsz], rstd[:qsz], gamma_bc[:qsz])
                        negmu = lnp.tile([128, 1], F32, tag="negmu")
                        nc.vector.tensor_mul(negmu[:qsz], mv[:qsz, 0:1], rstd[:qsz])
                        nc.vector.tensor_scalar_mul(negmu[:qsz], negmu[:qsz], -1.0)
                        g1 = lnp.tile([128, D], F32, tag="g1")
                        nc.scalar.activation(out=g1[:qsz], in_=ot[:qsz],
                                             func=AF.Identity,
                                             bias=negmu[:qsz], scale=rstd[:qsz])
                        gtok = gtk.tile([128, D], BF, tag="gtok")
                        nc.vector.tensor_add(gtok[:qsz], g1[:qsz], delta_bc[:qsz])
                        col = s * NT + c * 128
                        pst = ps_t.tile([128, DC, 128], BF, tag="pst")
                        for d in range(DC):
                            nc.tensor.transpose(pst[:, d, :qsz],
                                                gtok[:qsz, d * 128:(d + 1) * 128],
                                                ident[:qsz, :qsz])
                        nc.vector.tensor_copy(out=gT[:, :, col:col + qsz],
                                              in_=pst[:, :, :qsz])

                # ---- Q/K projections (e-major) ----
                for (W, OUT) in ((wq_De, QT), (wk_De, KT)):
                    for hh in range(HH):
                        for hf in range(2):
                            fsl = slice(hf * HALF, (hf + 1) * HALF)
                            ps = ps_mm.tile([128, HALF], F32, tag="proj_ps")
                            for d in range(DC):
                                nc.tensor.matmul(
                                    ps, W[d][:, hh * 128:(hh + 1) * 128],
                                    gT[:, d, fsl],
                                    start=(d == 0), stop=(d == DC - 1))
                            nc.scalar.copy(out=OUT[hh][:, fsl], in_=ps)

                # ---- attention per sample ----
                for s in range(SPC):
                    scol = slice(s * NT, (s + 1) * NT)
                    # token-major Q/K for this sample (batched transp-evac)
                    Qs = [atk.tile([128, D], BF, tag=f"Qs{c}", name=f"Qs{c}")
                          for c in range(2)]
                    Ks = [atk.tile([128, D], BF, tag=f"Ks{c}", name=f"Ks{c}")
                          for c in range(2)]
                    for (SRC, DST) in ((QT, Qs), (KT, Ks)):
                        for c in range(2):
                            qsz = QSZ[c]
                            col = s * NT + c * 128
                            pst = ps_t.tile([128, DC, 128], BF, tag="pst")
                            for hh in range(HH):
                                nc.tensor.transpose(pst[:qsz, hh, :],
                                                    SRC[hh][:, col:col + qsz], ident)
                            nc.vector.tensor_copy(
                                out=DST[c][:qsz, :].rearrange(
                                    "p (a b) -> p a b", a=DC),
                                in_=pst[:qsz, :, :])
                    # per head: S, exp+Z, P, T2^T; P^T via PE transpose; U1
                    for h in range(H):
                        hh, ho = h // 2, (h % 2) * 64
                        esl = slice(ho, ho + 64)
                        hsl = slice(h * 64, (h + 1) * 64)
                        Es = {}
                        for c in range(2):
                            qsz = QSZ[c]
                            col = s * NT + c * 128
                            ps = ps_at.tile([128, NT], F32, tag="psS")
                            nc.tensor.matmul(ps[:qsz], QT[hh][esl, col:col + qsz],
                                             KT[hh][esl, scol], start=True, stop=True)
                            E = atn.tile([128, NT], BF, tag="E")
                            Z = atn.tile([128, 1], F32, tag="Z")
                            nc.scalar.activation(out=E[:qsz], in_=ps[:qsz],
                                                 func=AF.Exp, scale=BETA,
                                                 accum_out=Z[:qsz])
                            Zi = atn.tile([128, 1], F32, tag="Zi")
                            nc.vector.reciprocal(Zi[:qsz], Z[:qsz])
                            nc.vector.tensor_scalar_mul(E[:qsz], E[:qsz], Zi[:qsz])
                            Es[c] = E
                        psT2 = ps_sm.tile([64, NT], F32, tag="ps_sm")
                        for c in range(2):
                            qsz = QSZ[c]
                            nc.tensor.matmul(psT2, Qs[c][:qsz, hsl], Es[c][:qsz],
                                             start=(c == 0), stop=(c == 1))
                        nc.vector.tensor_copy(out=T2T[hh][esl, scol], in_=psT2)
                        # P^T = transpose(P) per k-chunk, batched evac
                        PTs = {}
                        for kc in range(2):
                            ksz = QSZ[kc]
                            kcs = slice(kc * 128, kc * 128 + ksz)
                            pstp = ps_t.tile([128, 2, 128], BF, tag="pst")
                            for c in range(2):
                                qsz = QSZ[c]
                                nc.tensor.transpose(pstp[:ksz, c, :qsz],
                                                    Es[c][:qsz, kcs],
                                                    ident[:qsz, :qsz])
                            PT = atn.tile([128, 2, 128], BF, tag="PT")
                            nc.vector.tensor_copy(out=PT[:ksz], in_=pstp[:ksz])
                            PTs[kc] = PT
                        psU = ps_sm.tile([64, NT], F32, tag="ps_sm")
                        for kc in range(2):
                            ksz = QSZ[kc]
                            nc.tensor.matmul(
                                psU, Ks[kc][:ksz, hsl],
                                PTs[kc][:ksz].rearrange("p a b -> p (a b)")[:, :NT],
                                start=(kc == 0), stop=(kc == 1))
                        nc.vector.tensor_copy(out=U1T[hh][esl, scol], in_=psU)

                # ---- attention output projections ----
                for d in range(DC):
                    dsl = slice(d * 128, (d + 1) * 128)
                    for hf in range(2):
                        fsl = slice(hf * HALF, (hf + 1) * HALF)
                        ps1 = ps_mm.tile([128, HALF], F32, tag="proj_ps")
                        for hh in range(HH):
                            nc.tensor.matmul(ps1, wq_eD[hh][:, dsl], U1T[hh][:, fsl],
                                             start=(hh == 0), stop=(hh == HH - 1))
                        nc.vector.tensor_copy(out=UTc[d][:, fsl], in_=ps1)
                        ps2 = ps_mm.tile([128, HALF], F32, tag="proj_ps")
                        for hh in range(HH):
                            nc.tensor.matmul(ps2, wk_eD[hh][:, dsl], T2T[hh][:, fsl],
                                             start=(hh == 0), stop=(hh == HH - 1))
                        nc.vector.tensor_add(UTc[d][:, fsl], UTc[d][:, fsl], ps2)

                # ---- Hopfield ----
                for hf in range(2):
                    fsl = slice(hf * HALF, (hf + 1) * HALF)
                    hids = []
                    for m in range(MC):
                        xb = xip.tile([128, DC, 128], BF, tag="xib")
                        nc.sync.dma_start(out=xb, in_=d_xiblk[m])
                        ps = ps_mm.tile([128, HALF], F32, tag="proj_ps")
                        for d in range(DC):
                            nc.tensor.matmul(ps, xb[:, d, :], gT[:, d, fsl],
                                             start=(d == 0), stop=(d == DC - 1))
                        ht = hidp.tile([128, HALF], BF, tag=f"hid{m}", name=f"hid{m}")
                        nc.scalar.activation(out=ht, in_=ps, func=AF.Relu)
                        hids.append(ht)
                    for d in range(DC):
                        dsl = slice(d * 128, (d + 1) * 128)
                        ps = ps_mm.tile([128, HALF], F32, tag="proj_ps")
                        for m in range(MC):
                            nc.tensor.matmul(ps, xi_mD[m][:, dsl], hids[m],
                                             start=(m == 0), stop=(m == MC - 1))
                        nc.vector.tensor_add(UTc[d][:, fsl], UTc[d][:, fsl], ps)

                # ---- update: out += transpose(UTc), batched per (s,c) ----
                for s in range(SPC):
                    for c in range(2):
                        qsz = QSZ[c]
                        col = s * NT + c * 128
                        pst = ps_t.tile([128, DC, 128], BF, tag="pst")
                        for d in range(DC):
                            nc.tensor.transpose(pst[:qsz, d, :],
                                                UTc[d][:, col:col + qsz], ident)
                        ot = out_t[2 * s + c]
                        nc.vector.tensor_add(
                            ot[:qsz, :], ot[:qsz, :],
                            pst[:qsz, :, :].rearrange("p a b -> p (a b)"))

            if use_loop and n_iter > 1:
                with tc.For_i(0, n_iter, 1,
                              hint_engines=(mybir.EngineType.PE,
                                            mybir.EngineType.Activation,
                                            mybir.EngineType.DVE)) as _:
                    iteration_body()
            else:
                for _ in range(n_iter):
                    iteration_body()

        # ================= HEAD =================
        with tc.tile_pool(name="head", bufs=1) as hp, \
             tc.tile_pool(name="head_ps", bufs=2, space="PSUM") as hps:
            fcT_sb = ptiles(hp, DC, [128, NCLS], BF, "fcT")
            for i in range(DC):
                nc.sync.dma_start(out=fcT_sb[i], in_=d_fcwT[i * 128:(i + 1) * 128, :])
            lnw_sb = hp.tile([SPC, D], F32, tag="lnw")
            lnb_sb = hp.tile([SPC, D], F32, tag="lnb")
            fcb_sb = hp.tile([SPC, NCLS], F32, tag="fcb")
            nc.sync.dma_start(out=lnw_sb, in_=d_lnw[:, :])
            nc.sync.dma_start(out=lnb_sb, in_=d_lnb[:, :])
            nc.sync.dma_start(out=fcb_sb, in_=d_fcb[:, :])

            cls = hp.tile([SPC, D], F32, tag="cls")
            for s in range(SPC):
                nc.sync.dma_start(out=cls[s:s + 1, :],
                                  in_=out_t[2 * s + 1][CLS_ROW:CLS_ROW + 1, :])
            stats = hp.tile([SPC, 3, 6], F32, tag="hstats")
            mv = hp.tile([SPC, 2], F32, tag="hmv")
            for sg in range(3):
                nc.vector.bn_stats(out=stats[:, sg, :],
                                   in_=cls[:, sg * 256:(sg + 1) * 256])
            nc.vector.bn_aggr(out=mv, in_=stats)
            rstd = hp.tile([SPC, 1], F32, tag="hrstd")
            nc.scalar.activation(out=rstd, in_=mv[:, 1:2], func=AF.Sqrt,
                                 bias=eps_t[:SPC], scale=1.0)
            nc.vector.reciprocal(rstd, rstd)
            negmu = hp.tile([SPC, 1], F32, tag="hnegmu")
            nc.vector.tensor_mul(negmu, mv[:, 0:1], rstd)
            nc.vector.tensor_scalar_mul(negmu, negmu, -1.0)
            nhat = hp.tile([SPC, D], F32, tag="nhat")
            nc.scalar.activation(out=nhat, in_=cls, func=AF.Identity,
                                 bias=negmu, scale=rstd)
            nc.vector.tensor_mul(nhat, nhat, lnw_sb)
            chat = hp.tile([SPC, D], BF, tag="chat")
            nc.vector.tensor_add(chat, nhat, lnb_sb)
            cT = hp.tile([128, DC, SPC], BF, tag="cT")
            for d in range(DC):
                pst = hps.tile([128, SPC], BF, tag="hpst")
                nc.tensor.transpose(pst[:, :SPC], chat[:SPC, d * 128:(d + 1) * 128],
                                    ident[:SPC, :SPC])
                nc.scalar.copy(out=cT[:, d, :], in_=pst[:, :SPC])
            logits = hp.tile([SPC, NCLS], F32, tag="logits")
            for nh in range(2):
                nsl = slice(nh * 500, (nh + 1) * 500)
                psL = hps.tile([SPC, 500], F32, tag="psL")
                for d in range(DC):
                    nc.tensor.matmul(psL, cT[:, d, :], fcT_sb[d][:, nsl],
                                     start=(d == 0), stop=(d == DC - 1))
                nc.vector.tensor_add(logits[:, nsl], psL, fcb_sb[:, nsl])
            nc.sync.dma_start(out=d_y[:, :], in_=logits)

    nc.compile()
    return nc


def _patchify(x):
    b = x.shape[0]
    xx = x.reshape(b, 3, PATCH, PS, PATCH, PS)
    xx = xx.transpose(0, 2, 4, 3, 5, 1)  # b, ph, pw, ps1, ps2, c
    return xx.reshape(b, PATCH * PATCH, F)


def _bf(a):
    return np.ascontiguousarray(np.asarray(a, np.float32)).astype(ml_dtypes.bfloat16)


def _f32(a):
    return np.ascontiguousarray(np.asarray(a, np.float32))


def make_in_maps(x, emb_w, emb_b, cls_token, pos_emb, gamma, delta, wq, wk, xi,
                 ln_w, ln_b, fc_w, fc_b):
    x = np.asarray(x, np.float32)
    pos_emb = np.asarray(pos_emb, np.float32)
    emb_b = np.asarray(emb_b, np.float32)
    cls_token = np.asarray(cls_token, np.float32)
    wq = np.asarray(wq, np.float32)
    wk = np.asarray(wk, np.float32)
    xi = np.asarray(xi, np.float32)
    # token order: [patch 0..195, cls]
    posc = np.empty((NT, D), np.float32)
    posc[:NPATCH] = pos_emb[0, 1:] + np.asarray(emb_b, np.float32)[None, :]
    posc[NPATCH] = pos_emb[0, 0] + cls_token[0, 0]
    shared = {
        "embwT": _bf(np.asarray(emb_w, np.float32).T),
        "posc": _f32(posc),
        "wq_De": _bf(wq.transpose(1, 0, 2).reshape(D, D)),
        "wk_De": _bf(wk.transpose(1, 0, 2).reshape(D, D)),
        "wq_eD": _bf(wq.transpose(0, 2, 1).reshape(D, D)),
        "wk_eD": _bf(wk.transpose(0, 2, 1).reshape(D, D)),
        # [m, p, d, f] = xi[d*128+p, m*128+f]
        "xiblk": _bf(xi.reshape(DC, 128, MC, 128).transpose(2, 1, 0, 3)),
        "ximD": _bf(xi.T),
        "delta_bc": _bf(np.tile(np.asarray(delta, np.float32)[None, :], (128, 1))),
        "gamma_bc": _f32(np.full((128, 1), np.float32(gamma))),
        "ident_bf": _bf(np.eye(128)),
        "ones_bf": _bf(np.ones((1, 128))),
        "lnw_bc": _f32(np.tile(np.asarray(ln_w, np.float32)[None, :], (SPC, 1))),
        "lnb_bc": _f32(np.tile(np.asarray(ln_b, np.float32)[None, :], (SPC, 1))),
        "fcwT": _bf(np.asarray(fc_w, np.float32).T),
        "fcb_bc": _f32(np.tile(np.asarray(fc_b, np.float32)[None, :], (SPC, 1))),
    }
    in_maps = []
    for c in range(N_CORES):
        xs = x[c * SPC:(c + 1) * SPC]
        patches = _patchify(xs).reshape(SPC * NPATCH, F)
        m = dict(shared)
        m["pT"] = _bf(patches.T)
        in_maps.append(m)
    return in_maps


_NC_CACHE = None


def kernel(x, emb_w, emb_b, cls_token, pos_emb, gamma, delta, wq, wk, xi,
           ln_w, ln_b, fc_w, fc_b):
    global _NC_CACHE
    in_maps = make_in_maps(x, emb_w, emb_b, cls_token, pos_emb, gamma, delta,
                           wq, wk, xi, ln_w, ln_b, fc_w, fc_b)
    if _NC_CACHE is None:
        _NC_CACHE = build_nc()
    res = run_bass_kernel_spmd(_NC_CACHE, in_maps, core_ids=list(range(N_CORES)))
    return np.concatenate([r["y"] for r in res.results], axis=0)
